# revision 28
# baseline (speedup 1.0000x reference)
"""Trainium2 Bass kernel for a CQT (constant-Q transform) nn.Module.

Reference computation (per batch sample b, channel c):
    out[b, c, k, f, 0] = sum_t x[b, c, f*HOP + t] * w_re[k, t]
    out[b, c, k, f, 1] = sum_t x[b, c, f*HOP + t] * w_im[k, t]
where w_re/w_im are Hann-windowed complex exponentials with per-bin ragged
lengths (longest 11340 samples), HOP=512, 84 bins, 409 frames.

Strategy: data-parallel over the batch (1 sample per NeuronCore, 8 cores).
Per core the correlation is a banded matmul: the contraction axis t is split
into 89 chunks of 128; chunk c needs x samples x[(f + c//4)*512 + (c%4)*128 + r].
The signal is laid out once in SBUF as Xt[r, ch, rc, m] = x[ch, m*512+rc*128+r]
so every chunk's moving operand is just a 409-column slice of a resident tile.

Weight rows are interleaved (re_k, im_k) pairs sorted by descending window
length, so the rows active in a chunk are always a prefix. Rows 0..127
(bins 0..63) form accumulation group G1 (89 chunks); rows 128..167
(bins 64..83, windows <= 281 samples) form group G2 (3 chunks). The weight
matrix is stored column-compacted (only active rows per chunk), cutting the
weight DMA from 7.9 MB to 1.6 MB without changing the matmul cost.
"""

import math
from contextlib import ExitStack

import numpy as np

import concourse.bass as bass
import concourse.mybir as mybir
import concourse.tile as tile
from concourse import bacc
from concourse.bass_utils import run_bass_kernel_spmd

# ---- problem constants (hardcoded CQT spec) ----
SR = 22050
N_BINS = 84
BPO = 12
FMIN = 32.7
HOP = 512
B, C, T = 8, 2, 220500
N_CORES = 8

LMAX = 11340           # longest window
F = 409                # frames: 1 + (T - LMAX)//HOP
NCHUNK = 89            # ceil(LMAX/128) contraction chunks
MBLK = 432             # 512-sample blocks of x: (F-1)+(NCHUNK-1)//4+1 = 431, +1 pad for FP=410
FP = 410               # fp32r needs an even moving free dim; frame 409 is junk
NROWS = 2 * N_BINS     # interleaved (re, im) weight rows
G1ROWS = 128           # group 1 = rows 0..127  (bins 0..63)
G2ROWS = NROWS - G1ROWS  # 40 rows (bins 64..83)
import os as _os
WBLK0 = int(_os.environ.get("K_WBLK0", "1"))  # chunks in first weight block
WBLK = int(_os.environ.get("K_WBLK", "12"))   # chunks per later weight block
N_WARM = int(_os.environ.get("K_NWARM", "5"))  # PE warm-up matmuls

MM_DT = mybir.dt.float32r  # tensor-engine matmul dtype (full-rate fp32)

_PREP = None
_NC = None
LAST_RESULTS = None


def _params():
    """Host-side constants: compacted weight arrays + chunk geometry."""
    global _PREP
    if _PREP is not None:
        return _PREP

    Q = 1.0 / (2.0 ** (1.0 / BPO) - 1.0)
    freqs = FMIN * 2.0 ** (np.arange(N_BINS, dtype=np.float64) / BPO)
    lengths = np.round(Q * SR / freqs).astype(np.int64)
    assert int(lengths.max()) == LMAX

    t = np.arange(LMAX, dtype=np.float64)
    L = lengths.astype(np.float64)[:, None]
    mask = (t[None, :] < L).astype(np.float64)
    win = 0.5 * (1.0 - np.cos(2.0 * math.pi * t[None, :] / L)) * mask
    phase = (2.0 * math.pi / SR) * freqs[:, None] * t[None, :]
    w_re = (win * np.cos(phase)).astype(np.float32)
    w_im = (-win * np.sin(phase)).astype(np.float32)

    # rows 2k / 2k+1 = re_k / im_k; zero-pad time to NCHUNK*128
    W = np.zeros((NROWS, NCHUNK * 128), dtype=np.float32)
    W[0::2, :LMAX] = w_re
    W[1::2, :LMAX] = w_im
    WT = np.ascontiguousarray(W.T)  # (NCHUNK*128, NROWS)

    n_act = np.array([(lengths > 128 * c).sum() for c in range(NCHUNK)])
    assert n_act[0] == N_BINS and n_act[-1] >= 1
    mG1 = np.minimum(G1ROWS, 2 * n_act).astype(np.int64)
    G2C = math.ceil(int(lengths[G1ROWS // 2]) / 128)  # chunks needed by bin 64
    mG2 = (2 * n_act[:G2C] - G1ROWS).astype(np.int64)
    assert mG2[0] == G2ROWS and (mG2 > 0).all()

    base = np.zeros(NCHUNK + 1, dtype=np.int64)
    base[1:] = np.cumsum(mG1)
    SG1 = int(base[-1])
    g2base = np.zeros(G2C + 1, dtype=np.int64)
    g2base[1:] = np.cumsum(mG2)
    SG2 = int(g2base[-1])

    wg1 = np.zeros((128, SG1), dtype=np.float32)
    for c in range(NCHUNK):
        wg1[:, base[c]:base[c + 1]] = WT[128 * c:128 * (c + 1), :mG1[c]]
    wg2 = np.zeros((128, SG2), dtype=np.float32)
    for c in range(G2C):
        wg2[:, g2base[c]:g2base[c + 1]] = WT[128 * c:128 * (c + 1),
                                             G1ROWS:G1ROWS + mG2[c]]

    _PREP = dict(mG1=mG1, mG2=mG2, G2C=G2C, base=base, g2base=g2base,
                 SG1=SG1, SG2=SG2, wg1=wg1, wg2=wg2)
    return _PREP


def _build_nc():
    p = _params()
    mG1, mG2, G2C = p["mG1"], p["mG2"], p["G2C"]
    base, g2base, SG1, SG2 = p["base"], p["g2base"], p["SG1"], p["SG2"]

    nc = bacc.Bacc(None, target_bir_lowering=False)
    xt_d = nc.dram_tensor("xt", (C, 4, 128, MBLK), MM_DT, kind="ExternalInput")
    wg1_d = nc.dram_tensor("wg1", (128, SG1), MM_DT, kind="ExternalInput")
    wg2_d = nc.dram_tensor("wg2", (128, SG2), MM_DT, kind="ExternalInput")
    out_d = nc.dram_tensor("out", (C, NROWS, F), mybir.dt.float32,
                           kind="ExternalOutput")

    with ExitStack() as ctx:
        tc = ctx.enter_context(tile.TileContext(nc))
        xp = ctx.enter_context(tc.tile_pool(name="xp", bufs=1))
        wp = ctx.enter_context(tc.tile_pool(name="wp", bufs=1))
        op = ctx.enter_context(tc.tile_pool(name="op", bufs=1))
        pp = ctx.enter_context(tc.tile_pool(name="pp", bufs=1, space="PSUM"))

        # PE warm-up: dummy matmuls on never-written (garbage) SBUF keep the
        # PE busy through the HAM cold window while input DMAs run. Results
        # go to a scratch bank that is never read.
        warm_sb = xp.tile([128, 128], MM_DT, name="warm_sb", tag="warm_sb")
        warm_ps = pp.tile([128, 128], mybir.dt.float32, name="warm_ps",
                          tag="warm_ps")
        nc.vector.memset(warm_sb[:].bitcast(mybir.dt.float32), 0.0)
        for _ in range(N_WARM):
            nc.tensor.matmul(warm_ps[:, :], warm_sb[:, :], warm_sb[:, :],
                             start=True, stop=True, skip_group_check=True)

        # Input DMA plan: two parallel streams. The sync (HWDGE) queue carries
        # the channel-0 signal tiles first (they gate the first matmuls),
        # then half the weight blocks, then channel 1 (not needed until
        # halfway). The gpsimd (SWDGE) queue carries the other weight blocks,
        # starting with the tiny first block that gates matmul #1.
        wbounds = [0, WBLK0]
        while wbounds[-1] < NCHUNK:
            wbounds.append(min(wbounds[-1] + WBLK, NCHUNK))
        nblk = len(wbounds) - 1
        wtiles = []
        for b0, b1 in zip(wbounds[:-1], wbounds[1:]):
            cols = int(base[b1] - base[b0])
            wtiles.append(wp.tile([128, cols], MM_DT, name=f"w_{b0}",
                                  tag=f"w_{b0}"))
        wg2_sb = wp.tile([128, SG2], MM_DT, name="wg2_sb", tag="wg2_sb")
        xt_sb = {(ch, rc): xp.tile([128, MBLK], MM_DT, name=f"x_{ch}_{rc}",
                                   tag=f"x_{ch}_{rc}")
                 for ch in range(C) for rc in range(4)}

        def dma_w(i, eng):
            b0, b1 = wbounds[i], wbounds[i + 1]
            eng.dma_start(wtiles[i][:], wg1_d[:, int(base[b0]):int(base[b1])])

        # gpsimd stream: first block + every later odd-indexed block
        dma_w(0, nc.gpsimd)
        for i in range(1, nblk, 2):
            dma_w(i, nc.gpsimd)
        # sync stream: ch0 signal, small G2 weights, even blocks, then ch1
        nc.sync.dma_start(xt_sb[0, 0][:], xt_d[0, 0])
        nc.sync.dma_start(wg2_sb[:], wg2_d[:])
        for rc in range(1, 4):
            nc.sync.dma_start(xt_sb[0, rc][:], xt_d[0, rc])
        for i in range(2, nblk, 2):
            dma_w(i, nc.sync)
        for rc in range(4):
            nc.sync.dma_start(xt_sb[1, rc][:], xt_d[1, rc])

        def wblk_of(c):
            for b0, b1, wt in zip(wbounds[:-1], wbounds[1:], wtiles):
                if b0 <= c < b1:
                    return b0, wt
            raise AssertionError(c)

        for ch in range(C):
            ps1 = pp.tile([128, FP], mybir.dt.float32, name=f"ps1_{ch}",
                          tag=f"ps1_{ch}")
            ps2 = pp.tile([128, FP], mybir.dt.float32, name=f"ps2_{ch}",
                          tag=f"ps2_{ch}")
            for c in range(NCHUNK):
                j, rc = divmod(c, 4)
                b0, wtile = wblk_of(c)
                off = int(base[c] - base[b0])
                m = int(mG1[c])
                # ragged prefix accumulation: rows [mG1[c+1], mG1[c]) see
                # their last write before the group's nominal stop, which the
                # sim group checker can't express — data correctness comes
                # from the pending-zero mechanism (start=True on chunk 0
                # zeroes all 128 rows of the bank region).
                nc.tensor.matmul(
                    ps1[0:m, :], wtile[:, off:off + m],
                    xt_sb[ch, rc][:, j:j + FP],
                    start=(c == 0), stop=(c == NCHUNK - 1),
                    skip_group_check=True)
            # G2 last: its 3 matmuls overlap G1's copy + output DMA
            for c in range(G2C):
                j, rc = divmod(c, 4)
                m = int(mG2[c])
                nc.tensor.matmul(
                    ps2[0:m, :], wg2_sb[:, int(g2base[c]):int(g2base[c]) + m],
                    xt_sb[ch, rc][:, j:j + FP],
                    start=(c == 0), stop=(c == G2C - 1),
                    skip_group_check=True)
            o1 = op.tile([128, F], mybir.dt.float32, name=f"o1_{ch}",
                         tag=f"o1_{ch}")
            o2 = op.tile([G2ROWS, F], mybir.dt.float32, name=f"o2_{ch}",
                         tag=f"o2_{ch}")
            nc.vector.tensor_copy(o1[:], ps1[:, 0:F])
            nc.sync.dma_start(out_d[ch, 0:G1ROWS, :], o1[:])
            nc.vector.tensor_copy(o2[:], ps2[0:G2ROWS, 0:F])
            nc.sync.dma_start(out_d[ch, G1ROWS:NROWS, :], o2[:])
    nc.finalize()
    return nc


def get_nc():
    global _NC
    if _NC is None:
        _NC = _build_nc()
    return _NC


def _pack_x(xb):
    """(C, T) -> (C, 4, 128, MBLK) with xt[ch, rc, r, m] = x[ch, m*512+rc*128+r]."""
    xpad = np.zeros((C, MBLK * 512), dtype=np.float32)
    xpad[:, :T] = xb
    return np.ascontiguousarray(
        xpad.reshape(C, MBLK, 4, 128).transpose(0, 2, 3, 1))


def kernel(x):
    global LAST_RESULTS
    x = np.asarray(x, dtype=np.float32)
    assert x.shape == (B, C, T)
    p = _params()
    in_maps = [{"xt": _pack_x(x[b]), "wg1": p["wg1"], "wg2": p["wg2"]}
               for b in range(B)]
    nc = get_nc()
    res = run_bass_kernel_spmd(nc, in_maps, core_ids=list(range(N_CORES)))
    LAST_RESULTS = res
    out = np.empty((B, C, N_BINS, F, 2), dtype=np.float32)
    for b in range(B):
        raw = np.asarray(res.results[b]["out"])  # (C, NROWS, F)
        out[b] = raw.reshape(C, N_BINS, 2, F).transpose(0, 1, 3, 2)
    return out


# revision 31
# speedup vs baseline: 19389.1387x; 19389.1387x over previous
"""Trainium2 Bass kernel for a CQT (constant-Q transform) nn.Module.

Reference computation (per batch sample b, channel c):
    out[b, c, k, f, 0] = sum_t x[b, c, f*HOP + t] * w_re[k, t]
    out[b, c, k, f, 1] = sum_t x[b, c, f*HOP + t] * w_im[k, t]
where w_re/w_im are Hann-windowed complex exponentials with per-bin ragged
lengths (longest 11340 samples), HOP=512, 84 bins, 409 frames.

Strategy: data-parallel over the batch (1 sample per NeuronCore, 8 cores).
Per core the correlation is a banded matmul: the contraction axis t is split
into 89 chunks of 128; chunk c needs x samples x[(f + c//4)*512 + (c%4)*128 + r].
The signal is laid out once in SBUF as Xt[r, ch, rc, m] = x[ch, m*512+rc*128+r]
so every chunk's moving operand is a 410-column slice of a resident tile
(410 = 409 frames padded to the even count fp32r requires).

Matmuls run in float32r: full fp32 data, 1 cycle/row on the PE when the
moving dim >= 256 (4x faster than plain fp32; confirmed on HW via a For_i
repeat-loop wall-clock probe). End-to-end relative error vs the fp32
reference is ~1.4e-4.

Weight rows are interleaved (re_k, im_k) pairs sorted by descending window
length, so the rows active in a chunk are always a prefix. Rows 0..127
(bins 0..63) form accumulation group G1 (89 chunks); rows 128..167
(bins 64..83, windows <= 281 samples) form group G2 (3 chunks). The weight
matrix is stored column-compacted (only active rows per chunk), cutting the
weight DMA from 7.9 MB to 1.6 MB without changing the matmul cost.
"""

import math
import os as _os
from contextlib import ExitStack

import numpy as np

import concourse.bass as bass
import concourse.mybir as mybir
import concourse.tile as tile
from concourse import bacc
from concourse.bass_utils import run_bass_kernel_spmd

# ---- problem constants (hardcoded CQT spec) ----
SR = 22050
N_BINS = 84
BPO = 12
FMIN = 32.7
HOP = 512
B, C, T = 8, 2, 220500
N_CORES = 8

LMAX = 11340           # longest window
F = 409                # frames: 1 + (T - LMAX)//HOP
NCHUNK = 89            # ceil(LMAX/128) contraction chunks
MBLK = 432             # 512-sample blocks of x: (F-1)+(NCHUNK-1)//4+1 = 431, +1 pad for FP=410
FP = 410               # fp32r needs an even moving free dim; frame 409 is junk
NROWS = 2 * N_BINS     # interleaved (re, im) weight rows
G1ROWS = 128           # group 1 = rows 0..127  (bins 0..63)
G2ROWS = NROWS - G1ROWS  # 40 rows (bins 64..83)
WBLK0 = int(_os.environ.get("K_WBLK0", "1"))  # chunks in first weight block
WBLK = int(_os.environ.get("K_WBLK", "12"))   # chunks per later weight block
N_WARM = int(_os.environ.get("K_NWARM", "5"))  # PE warm-up matmuls

MM_DT = mybir.dt.float32r  # tensor-engine matmul dtype (full-rate fp32)

_PREP = None
_NC = None
LAST_RESULTS = None


def _params():
    """Host-side constants: compacted weight arrays + chunk geometry."""
    global _PREP
    if _PREP is not None:
        return _PREP

    Q = 1.0 / (2.0 ** (1.0 / BPO) - 1.0)
    freqs = FMIN * 2.0 ** (np.arange(N_BINS, dtype=np.float64) / BPO)
    lengths = np.round(Q * SR / freqs).astype(np.int64)
    assert int(lengths.max()) == LMAX

    t = np.arange(LMAX, dtype=np.float64)
    L = lengths.astype(np.float64)[:, None]
    mask = (t[None, :] < L).astype(np.float64)
    win = 0.5 * (1.0 - np.cos(2.0 * math.pi * t[None, :] / L)) * mask
    phase = (2.0 * math.pi / SR) * freqs[:, None] * t[None, :]
    w_re = (win * np.cos(phase)).astype(np.float32)
    w_im = (-win * np.sin(phase)).astype(np.float32)

    # rows 2k / 2k+1 = re_k / im_k; zero-pad time to NCHUNK*128
    W = np.zeros((NROWS, NCHUNK * 128), dtype=np.float32)
    W[0::2, :LMAX] = w_re
    W[1::2, :LMAX] = w_im
    WT = np.ascontiguousarray(W.T)  # (NCHUNK*128, NROWS)

    n_act = np.array([(lengths > 128 * c).sum() for c in range(NCHUNK)])
    assert n_act[0] == N_BINS and n_act[-1] >= 1
    mG1 = np.minimum(G1ROWS, 2 * n_act).astype(np.int64)
    G2C = math.ceil(int(lengths[G1ROWS // 2]) / 128)  # chunks needed by bin 64
    mG2 = (2 * n_act[:G2C] - G1ROWS).astype(np.int64)
    assert mG2[0] == G2ROWS and (mG2 > 0).all()

    base = np.zeros(NCHUNK + 1, dtype=np.int64)
    base[1:] = np.cumsum(mG1)
    SG1 = int(base[-1])
    g2base = np.zeros(G2C + 1, dtype=np.int64)
    g2base[1:] = np.cumsum(mG2)
    SG2 = int(g2base[-1])

    wg1 = np.zeros((128, SG1), dtype=np.float32)
    for c in range(NCHUNK):
        wg1[:, base[c]:base[c + 1]] = WT[128 * c:128 * (c + 1), :mG1[c]]
    wg2 = np.zeros((128, SG2), dtype=np.float32)
    for c in range(G2C):
        wg2[:, g2base[c]:g2base[c + 1]] = WT[128 * c:128 * (c + 1),
                                             G1ROWS:G1ROWS + mG2[c]]

    _PREP = dict(mG1=mG1, mG2=mG2, G2C=G2C, base=base, g2base=g2base,
                 SG1=SG1, SG2=SG2, wg1=wg1, wg2=wg2)
    return _PREP


def _build_nc():
    p = _params()
    mG1, mG2, G2C = p["mG1"], p["mG2"], p["G2C"]
    base, g2base, SG1, SG2 = p["base"], p["g2base"], p["SG1"], p["SG2"]

    nc = bacc.Bacc(None, target_bir_lowering=False)
    xt_d = nc.dram_tensor("xt", (C, 4, 128, MBLK), MM_DT, kind="ExternalInput")
    wg1_d = nc.dram_tensor("wg1", (128, SG1), MM_DT, kind="ExternalInput")
    wg2_d = nc.dram_tensor("wg2", (128, SG2), MM_DT, kind="ExternalInput")
    out_d = nc.dram_tensor("out", (C, NROWS, F), mybir.dt.float32,
                           kind="ExternalOutput")

    with ExitStack() as ctx:
        tc = ctx.enter_context(tile.TileContext(nc))
        xp = ctx.enter_context(tc.tile_pool(name="xp", bufs=1))
        wp = ctx.enter_context(tc.tile_pool(name="wp", bufs=1))
        op = ctx.enter_context(tc.tile_pool(name="op", bufs=1))
        pp = ctx.enter_context(tc.tile_pool(name="pp", bufs=1, space="PSUM"))

        # PE warm-up: dummy matmuls on never-written (garbage) SBUF keep the
        # PE busy through the HAM cold window while input DMAs run. Results
        # go to a scratch bank that is never read.
        warm_sb = xp.tile([128, 128], MM_DT, name="warm_sb", tag="warm_sb")
        warm_ps = pp.tile([128, 128], mybir.dt.float32, name="warm_ps",
                          tag="warm_ps")
        nc.vector.memset(warm_sb[:].bitcast(mybir.dt.float32), 0.0)
        for _ in range(N_WARM):
            nc.tensor.matmul(warm_ps[:, :], warm_sb[:, :], warm_sb[:, :],
                             start=True, stop=True, skip_group_check=True)

        # Input DMA plan: two parallel streams. The sync (HWDGE) queue carries
        # the channel-0 signal tiles first (they gate the first matmuls),
        # then half the weight blocks, then channel 1 (not needed until
        # halfway). The gpsimd (SWDGE) queue carries the other weight blocks,
        # starting with the tiny first block that gates matmul #1.
        wbounds = [0, WBLK0]
        while wbounds[-1] < NCHUNK:
            wbounds.append(min(wbounds[-1] + WBLK, NCHUNK))
        nblk = len(wbounds) - 1
        wtiles = []
        for b0, b1 in zip(wbounds[:-1], wbounds[1:]):
            cols = int(base[b1] - base[b0])
            wtiles.append(wp.tile([128, cols], MM_DT, name=f"w_{b0}",
                                  tag=f"w_{b0}"))
        wg2_sb = wp.tile([128, SG2], MM_DT, name="wg2_sb", tag="wg2_sb")
        xt_sb = {(ch, rc): xp.tile([128, MBLK], MM_DT, name=f"x_{ch}_{rc}",
                                   tag=f"x_{ch}_{rc}")
                 for ch in range(C) for rc in range(4)}

        def dma_w(i, eng):
            b0, b1 = wbounds[i], wbounds[i + 1]
            eng.dma_start(wtiles[i][:], wg1_d[:, int(base[b0]):int(base[b1])])

        # gpsimd stream: first block + every later odd-indexed block
        dma_w(0, nc.gpsimd)
        for i in range(1, nblk, 2):
            dma_w(i, nc.gpsimd)
        # sync stream: ch0 signal, small G2 weights, even blocks, then ch1
        nc.sync.dma_start(xt_sb[0, 0][:], xt_d[0, 0])
        nc.sync.dma_start(wg2_sb[:], wg2_d[:])
        for rc in range(1, 4):
            nc.sync.dma_start(xt_sb[0, rc][:], xt_d[0, rc])
        for i in range(2, nblk, 2):
            dma_w(i, nc.sync)
        for rc in range(4):
            nc.sync.dma_start(xt_sb[1, rc][:], xt_d[1, rc])

        def wblk_of(c):
            for b0, b1, wt in zip(wbounds[:-1], wbounds[1:], wtiles):
                if b0 <= c < b1:
                    return b0, wt
            raise AssertionError(c)

        for ch in range(C):
            ps1 = pp.tile([128, FP], mybir.dt.float32, name=f"ps1_{ch}",
                          tag=f"ps1_{ch}")
            ps2 = pp.tile([128, FP], mybir.dt.float32, name=f"ps2_{ch}",
                          tag=f"ps2_{ch}")
            for c in range(NCHUNK):
                j, rc = divmod(c, 4)
                b0, wtile = wblk_of(c)
                off = int(base[c] - base[b0])
                m = int(mG1[c])
                # ragged prefix accumulation: rows [mG1[c+1], mG1[c]) see
                # their last write before the group's nominal stop, which the
                # sim group checker can't express — data correctness comes
                # from the pending-zero mechanism (start=True on chunk 0
                # zeroes all 128 rows of the bank region).
                nc.tensor.matmul(
                    ps1[0:m, :], wtile[:, off:off + m],
                    xt_sb[ch, rc][:, j:j + FP],
                    start=(c == 0), stop=(c == NCHUNK - 1),
                    skip_group_check=True)
            # G2 last: its 3 matmuls overlap G1's copy + output DMA
            for c in range(G2C):
                j, rc = divmod(c, 4)
                m = int(mG2[c])
                nc.tensor.matmul(
                    ps2[0:m, :], wg2_sb[:, int(g2base[c]):int(g2base[c]) + m],
                    xt_sb[ch, rc][:, j:j + FP],
                    start=(c == 0), stop=(c == G2C - 1),
                    skip_group_check=True)
            o1 = op.tile([128, F], mybir.dt.float32, name=f"o1_{ch}",
                         tag=f"o1_{ch}")
            o2 = op.tile([G2ROWS, F], mybir.dt.float32, name=f"o2_{ch}",
                         tag=f"o2_{ch}")
            nc.vector.tensor_copy(o1[:], ps1[:, 0:F])
            nc.sync.dma_start(out_d[ch, 0:G1ROWS, :], o1[:])
            nc.vector.tensor_copy(o2[:], ps2[0:G2ROWS, 0:F])
            nc.sync.dma_start(out_d[ch, G1ROWS:NROWS, :], o2[:])
    nc.finalize()
    return nc


def get_nc():
    global _NC
    if _NC is None:
        _NC = _build_nc()
    return _NC


def _pack_x(xb):
    """(C, T) -> (C, 4, 128, MBLK) with xt[ch, rc, r, m] = x[ch, m*512+rc*128+r]."""
    xpad = np.zeros((C, MBLK * 512), dtype=np.float32)
    xpad[:, :T] = xb
    return np.ascontiguousarray(
        xpad.reshape(C, MBLK, 4, 128).transpose(0, 2, 3, 1))


def kernel(x):
    global LAST_RESULTS
    x = np.asarray(x, dtype=np.float32)
    assert x.shape == (B, C, T)
    p = _params()
    in_maps = [{"xt": _pack_x(x[b]), "wg1": p["wg1"], "wg2": p["wg2"]}
               for b in range(B)]
    nc = get_nc()
    res = run_bass_kernel_spmd(nc, in_maps, core_ids=list(range(N_CORES)))
    LAST_RESULTS = res
    out = np.empty((B, C, N_BINS, F, 2), dtype=np.float32)
    for b in range(B):
        raw = np.asarray(res.results[b]["out"])  # (C, NROWS, F)
        out[b] = raw.reshape(C, N_BINS, 2, F).transpose(0, 1, 3, 2)
    return out
